# revision 28
# baseline (speedup 1.0000x reference)
"""Trainium2 Bass kernel for nn_MultiHeadAttention (B=4, T=2048, D=1024, H=16).

Sharding: 8 cores = 4 batches x 2 query-halves. Each core runs the full
attention for its 1024 queries against all 2048 keys (all 16 heads), so no
cross-core communication is needed; the host only concatenates the 8 output
slices. Odd cores receive a row-permuted x (their query half first) so the
same program runs on every core; attention is permutation-invariant over keys.

v2: fully SBUF-resident (no DRAM spills), all matmul operands bf16
(single-pass PE, half-cost LDWEIGHTS), Q/K generation interleaved with
attention pair-by-pair to keep the tensor engine stream dense (PE p-state
ramps to full clock only under sustained busy). PSUM budget: 3x[128,1024]
rotating "s" tiles (S logits + QK-gen + dn + rbc) + 1x[128,1024] "ops"
(per-pair O accumulator) = exactly 8 banks.

  x^T (d on partitions, bf16)  ->  Q^T, K^T (feature-major), V (token-major)
  S^T[k, q] = K_h^T.T @ Q_h^T   (PE, head pairs packed in partition halves)
  P^T = exp(S^T / 8)            (ACT, straight out of PSUM, bf16)
  O^T[d, q] += V_tile.T @ P^T   (PE, two heads packed via column groups)
  denom = ones.T @ sum_k P^T    (DVE accumulate + PE partition-sum)
  out[t, :] = (O^T / denom).T @ W_out + b_out
"""

import sys

sys.path.insert(0, "/opt/trn_rl_repo")

import numpy as np

B, T, D = 4, 2048, 1024
H, HD = 16, 64
NCORES = 8
TQ = T // 2  # queries per core
NP = 128
KT = T // NP  # 16 key tiles
DC = D // NP  # 8 d_model chunks
PAIRS = H // 2  # 8 head pairs; pair p owns features [128p, 128p+128)

_CACHE = {}


def _build():
    import concourse.bacc as bacc
    import concourse.tile as tile
    from concourse import masks, mybir

    F32 = mybir.dt.float32
    F32R = mybir.dt.float32r
    BF16 = mybir.dt.bfloat16
    AF = mybir.ActivationFunctionType

    nc = bacc.Bacc("TRN2", target_bir_lowering=False, debug=False,
                   num_devices=NCORES)
    x_io = nc.dram_tensor("x", [T, D], F32, kind="ExternalInput").ap()
    wqkv_io = nc.dram_tensor("wqkv", [D, 3 * D], F32, kind="ExternalInput").ap()
    bqkv_io = nc.dram_tensor("bqkv", [3 * D], F32, kind="ExternalInput").ap()
    wout_io = nc.dram_tensor("wout", [D, D], F32, kind="ExternalInput").ap()
    bout_io = nc.dram_tensor("bout", [D], F32, kind="ExternalInput").ap()
    out_io = nc.dram_tensor("out", [TQ, D], F32, kind="ExternalOutput").ap()

    bq_col = bqkv_io.rearrange("(n o) -> n o", o=1)  # [3072, 1]
    bq_row = bqkv_io.rearrange("(o n) -> o n", o=1)  # [1, 3072]
    bout_row = bout_io.rearrange("(o n) -> o n", o=1)  # [1, 1024]
    wq_3d = wqkv_io.rearrange("(dc pp) e -> pp dc e", dc=DC)  # [128, 8, 3072]

    with tile.TileContext(nc) as tc:
        with (
            tc.tile_pool(name="const", bufs=1) as cpool,
            tc.tile_pool(name="big", bufs=1) as big_pool,
            tc.tile_pool(name="otres", bufs=1) as ot_pool,
        ):
            ident_b = cpool.tile([NP, NP], BF16, name="ident_b")
            masks.make_identity(nc, ident_b[:])
            # persistent SBUF tensors (bf16)
            xT = big_pool.tile([NP, DC, T], BF16, name="xT")       # 32KB/part
            v_all = big_pool.tile([NP, KT, D], BF16, name="v_all")  # 32KB
            wv_sb = big_pool.tile([NP, DC, D], BF16, name="wv_sb")  # 16KB
            wout_sb = big_pool.tile([NP, PAIRS, D], BF16, name="wout_sb")  # 16KB

            # ---- Stage A: x -> x^T (bf16), via PE f32r transpose ----
            with (
                nc.named_scope("xT"),
                tc.tile_pool(name="stage", bufs=3) as stage_pool,
                tc.tile_pool(name="tr_ps", bufs=3, space="PSUM") as tr_pool,
            ):
                for j in range(KT):
                    xt = stage_pool.tile([NP, D], F32, name=f"xt{j}", tag="stg")
                    nc.sync.dma_start(xt[:], x_io[j * NP:(j + 1) * NP, :])
                    xb = stage_pool.tile([NP, D], BF16, name=f"xb{j}", tag="xb")
                    nc.vector.tensor_copy(xb[:], xt[:])
                    ps = tr_pool.tile([NP, D], BF16, name=f"trps{j}", tag="trps")
                    for dc in range(DC):
                        nc.tensor.transpose(
                            ps[:, dc * NP:(dc + 1) * NP],
                            xb[:, dc * NP:(dc + 1) * NP], ident_b[:])
                    nc.vector.tensor_copy(
                        xT[:, :, j * NP:(j + 1) * NP],
                        ps.rearrange("p (dc t) -> p dc t", dc=DC))

                # W_v load + cast (reuses stage pool)
                for dc in range(DC):
                    wtmp = stage_pool.tile([NP, D], F32, name=f"wvl{dc}",
                                           tag="stg")
                    nc.sync.dma_start(
                        wtmp[:], wqkv_io[dc * NP:(dc + 1) * NP, 2 * D:3 * D])
                    nc.vector.tensor_copy(wv_sb[:, dc, :], wtmp[:])

                # W_out load + cast (DVE work lands during V gen / attn)
                for p in range(PAIRS):
                    wtmp = stage_pool.tile([NP, D], F32, name=f"wol{p}",
                                           tag="stg")
                    nc.sync.dma_start(wtmp[:], wout_io[p * NP:(p + 1) * NP, :])
                    nc.vector.tensor_copy(wout_sb[:, p, :], wtmp[:])

            ones_col_b = cpool.tile([NP, 1], BF16, name="ones_col_b")
            nc.vector.memset(ones_col_b[:], 1.0)
            ones_row_b = cpool.tile([1, NP], BF16, name="ones_row_b")
            nc.vector.memset(ones_row_b[:], 1.0)
            # ind8[j][h, c] = 1.0 iff head h of pair j%4 owns column c
            ones64 = cpool.tile([1, 64], F32, name="ones64")
            nc.vector.memset(ones64[:], 1.0)
            ind8 = []
            for j in range(4):
                t = cpool.tile([8, NP], F32, name=f"ind8_{j}")
                nc.vector.memset(t[:], 0.0)
                nc.sync.dma_start(t[2 * j:2 * j + 1, 0:64], ones64[:])
                nc.sync.dma_start(t[2 * j + 1:2 * j + 2, 64:NP], ones64[:])
                tr = cpool.tile([8, NP], F32R, name=f"ind8r_{j}")
                nc.vector.tensor_copy(tr[:], t[:])
                ind8.append(tr)

            # b_v and b_out broadcast to [128, D] bf16 via K=1 ones matmul
            brow_f = cpool.tile([1, D], F32, name="brow_f")
            brow_b = cpool.tile([1, D], BF16, name="brow_b")
            bv_bc = cpool.tile([NP, D], BF16, name="bv_bc")
            bo_bc = cpool.tile([NP, D], BF16, name="bo_bc")
            with tc.tile_pool(name="bc_ps", bufs=2, space="PSUM") as bc_ps_pool:
                for dst, src in ((bv_bc, bq_row[:, 2 * D:3 * D]),
                                 (bo_bc, bout_row[:])):
                    nc.sync.dma_start(brow_f[:], src)
                    nc.vector.tensor_copy(brow_b[:], brow_f[:])
                    for c in range(2):
                        ps = bc_ps_pool.tile([NP, 512], F32, name="bcps",
                                             tag="bcps")
                        nc.tensor.matmul(ps[:], ones_row_b[:],
                                         brow_b[:, c * 512:(c + 1) * 512])
                        nc.vector.tensor_copy(dst[:, c * 512:(c + 1) * 512],
                                              ps[:])


            recip_in = [cpool.tile([8, TQ], F32, name=f"recip_in{h}")
                        for h in range(2)]
            recip_r = cpool.tile([8, TQ], F32R, name="recip_r")

            # ---- Stage B1: V (token-major, bf16) ----
            with (
                nc.named_scope("vgen"),
                tc.tile_pool(name="v_ps", bufs=4, space="PSUM") as v_ps_pool,
            ):
                for ti in range(KT):
                    pss = [v_ps_pool.tile([NP, 512], F32, name=f"vps{ti}_{c}",
                                          tag="vps") for c in range(2)]
                    for dc in range(DC):
                        for c in range(2):
                            nc.tensor.matmul(
                                pss[c][:], xT[:, dc, ti * NP:(ti + 1) * NP],
                                wv_sb[:, dc, c * 512:(c + 1) * 512],
                                start=(dc == 0), stop=(dc == DC - 1))
                    for c in range(2):
                        nc.vector.tensor_add(
                            v_all[:, ti, c * 512:(c + 1) * 512], pss[c][:],
                            bv_bc[:, c * 512:(c + 1) * 512])

            # ---- Stage B2/C: per-pair QK gen interleaved with attention ----
            oT = [ot_pool.tile([NP, TQ], BF16, name=f"oT{p}")
                  for p in range(PAIRS)]
            with (
                nc.named_scope("attn"),
                tc.tile_pool(name="wqk_st", bufs=1) as wqk_st_pool,
                tc.tile_pool(name="wqk", bufs=2) as wqk_pool,
                tc.tile_pool(name="bias", bufs=4) as bias_pool,
                tc.tile_pool(name="qt", bufs=3) as qt_pool,
                tc.tile_pool(name="kt", bufs=3) as kt_pool,
                tc.tile_pool(name="pt", bufs=3) as pt_pool,
                tc.tile_pool(name="acc", bufs=3) as acc_pool,
                tc.tile_pool(name="dnr", bufs=2) as dnr_pool,
                tc.tile_pool(name="s_ps", bufs=3, space="PSUM") as s_pool,
                tc.tile_pool(name="o_ps", bufs=1, space="PSUM") as o_pool,
            ):
                qts, kts, wqks, bqks = {}, {}, {}, {}

                def emit_gen_w(p):
                    # W_q / W_k tiles for pair p: load f32, cast bf16
                    wst = wqk_st_pool.tile([NP, DC, 2 * NP], F32,
                                           name=f"wst{p}", tag="wst")
                    nc.sync.dma_start(wst[:, :, 0:NP],
                                      wq_3d[:, :, p * NP:(p + 1) * NP])
                    nc.sync.dma_start(wst[:, :, NP:2 * NP],
                                      wq_3d[:, :, D + p * NP:D + (p + 1) * NP])
                    wqk = wqk_pool.tile([NP, DC, 2 * NP], BF16,
                                        name=f"wqk{p}", tag="wqk")
                    nc.vector.tensor_copy(wqk[:], wst[:])
                    bq = bias_pool.tile([NP, 1], F32, name=f"bq{p}", tag="bias")
                    nc.sync.dma_start(bq[:], bq_col[p * NP:(p + 1) * NP, :])
                    bk = bias_pool.tile([NP, 1], F32, name=f"bk{p}", tag="bias")
                    nc.sync.dma_start(bk[:],
                                      bq_col[D + p * NP:D + (p + 1) * NP, :])
                    wqks[p] = wqk
                    bqks[p] = (bq, bk)
                    qts[p] = qt_pool.tile([NP, TQ], BF16, name=f"qt{p}",
                                          tag="qt")
                    kts[p] = kt_pool.tile([NP, T], BF16, name=f"kt{p}",
                                          tag="kt")

                def emit_gen_q(p):
                    qps = s_pool.tile([NP, TQ], F32, name=f"qps{p}", tag="s")
                    for dc in range(DC):
                        for c in range(2):
                            nc.tensor.matmul(
                                qps[:, c * 512:(c + 1) * 512],
                                wqks[p][:, dc, 0:NP],
                                xT[:, dc, c * 512:(c + 1) * 512],
                                start=(dc == 0), stop=(dc == DC - 1))
                    nc.vector.tensor_scalar_add(qts[p][:], qps[:],
                                                bqks[p][0][:])

                def emit_gen_k(p, half):
                    kps = s_pool.tile([NP, TQ], F32, name=f"kps{p}_{half}",
                                      tag="s")
                    for dc in range(DC):
                        for c in range(2):
                            nc.tensor.matmul(
                                kps[:, c * 512:(c + 1) * 512],
                                wqks[p][:, dc, NP:2 * NP],
                                xT[:, dc,
                                   half * TQ + c * 512:half * TQ + (c + 1) * 512],
                                start=(dc == 0), stop=(dc == DC - 1))
                    nc.vector.tensor_scalar_add(
                        kts[p][:, half * TQ:(half + 1) * TQ], kps[:],
                        bqks[p][1][:])
                    if half == 1:
                        del wqks[p], bqks[p]

                def emit_recip(half):
                    with nc.allow_low_precision(reason="f32r denominators"):
                        nc.vector.reciprocal(recip_r[:], recip_in[half][:])

                def emit_rbc(half):
                    for p in range(4 * half, 4 * half + 4):
                        rbc = s_pool.tile([NP, TQ], F32, name=f"rbc{p}",
                                          tag="s")
                        for c in range(2):
                            nc.tensor.matmul(
                                rbc[:, c * 512:(c + 1) * 512],
                                ind8[p % 4][:],
                                recip_r[:, c * 512:(c + 1) * 512])
                        nc.vector.tensor_mul(oT[p][:], oT[p][:], rbc[:])

                def emit_attn(p, hooks):
                    qt, kt = qts.pop(p), kts.pop(p)
                    ops = o_pool.tile([NP, TQ], F32, name=f"ops{p}", tag="ops")
                    accA = acc_pool.tile([NP, TQ], BF16, name=f"accA{p}",
                                         tag="acc")
                    accB = acc_pool.tile([NP, TQ], BF16, name=f"accB{p}",
                                         tag="acc")
                    for i in range(KT):
                        sA = s_pool.tile([NP, TQ], F32, name=f"sA{p}_{i}",
                                         tag="s")
                        sB = s_pool.tile([NP, TQ], F32, name=f"sB{p}_{i}",
                                         tag="s")
                        for c in range(2):
                            nc.tensor.matmul(
                                sA[:, c * 512:(c + 1) * 512],
                                kt[0:HD, i * NP:(i + 1) * NP],
                                qt[0:HD, c * 512:(c + 1) * 512])
                        for c in range(2):
                            nc.tensor.matmul(
                                sB[:, c * 512:(c + 1) * 512],
                                kt[HD:NP, i * NP:(i + 1) * NP],
                                qt[HD:NP, c * 512:(c + 1) * 512])
                        pA = pt_pool.tile([NP, TQ], BF16, name=f"pA{p}_{i}",
                                          tag="pt")
                        pB = pt_pool.tile([NP, TQ], BF16, name=f"pB{p}_{i}",
                                          tag="pt")
                        nc.scalar.activation(pA[:], sA[:], AF.Exp, scale=0.125)
                        nc.scalar.activation(pB[:], sB[:], AF.Exp, scale=0.125)
                        if i == 0:
                            nc.vector.tensor_copy(accA[:], pA[:])
                            nc.vector.tensor_copy(accB[:], pB[:])
                        else:
                            nc.vector.tensor_add(accA[:], accA[:], pA[:])
                            nc.vector.tensor_add(accB[:], accB[:], pB[:])
                        # col-packed heads share PSUM banks; the sim's
                        # bank-granular group check false-positives here
                        for c in range(2):
                            nc.tensor.matmul(
                                ops[0:HD, c * 512:(c + 1) * 512],
                                v_all[:, i, p * NP:p * NP + HD],
                                pA[:, c * 512:(c + 1) * 512],
                                start=(i == 0), stop=(i == KT - 1),
                                skip_group_check=True)
                        for c in range(2):
                            nc.tensor.matmul(
                                ops[HD:NP, c * 512:(c + 1) * 512],
                                v_all[:, i, p * NP + HD:(p + 1) * NP],
                                pB[:, c * 512:(c + 1) * 512],
                                start=(i == 0), stop=(i == KT - 1),
                                skip_group_check=True)
                        if i in hooks:
                            hooks[i]()

                    nc.vector.tensor_copy(oT[p][:], ops[:])
                    # denominators: partition-sum of acc via ones matmul
                    for h, acc in ((0, accA), (1, accB)):
                        dn = s_pool.tile([NP, TQ], F32, name=f"dn{p}_{h}",
                                         tag="s")
                        for c in range(2):
                            nc.tensor.matmul(
                                dn[0:1, c * 512:(c + 1) * 512], ones_col_b[:],
                                acc[:, c * 512:(c + 1) * 512])
                        dnr = dnr_pool.tile([1, TQ], F32, name=f"dnr{p}_{h}",
                                            tag="dnr")
                        nc.vector.tensor_copy(dnr[:], dn[0:1, :])
                        r, rr = divmod(2 * p + h, 8)
                        nc.sync.dma_start(recip_in[r][rr:rr + 1, :], dnr[:])

                def emit_gen(p):
                    emit_gen_w(p)
                    emit_gen_q(p)
                    emit_gen_k(p, 0)
                    emit_gen_k(p, 1)

                emit_gen(0)
                emit_gen(1)
                for p in range(PAIRS):
                    emit_attn(p, {})
                    if p + 2 < PAIRS:
                        emit_gen(p + 2)
                    if p == 3:
                        emit_recip(0)
                    elif p == 4:
                        emit_rbc(0)
                emit_recip(1)
                emit_rbc(1)

            # ---- Stage D: out projection ----
            with (
                nc.named_scope("outproj"),
                tc.tile_pool(name="f_ps", bufs=4, space="PSUM") as f_ps_pool,
                tc.tile_pool(name="f_sb", bufs=3) as f_sb_pool,
            ):
                for tj in range(TQ // NP):
                    fsb = f_sb_pool.tile([NP, D], F32, name=f"fsb{tj}",
                                         tag="fsb")
                    pss = [f_ps_pool.tile([NP, 512], F32, name=f"fps{tj}_{c}",
                                          tag="fps") for c in range(2)]
                    for p in range(PAIRS):
                        for c in range(2):
                            nc.tensor.matmul(
                                pss[c][:], oT[p][:, tj * NP:(tj + 1) * NP],
                                wout_sb[:, p, c * 512:(c + 1) * 512],
                                start=(p == 0), stop=(p == PAIRS - 1))
                    for c in range(2):
                        nc.vector.tensor_add(
                            fsb[:, c * 512:(c + 1) * 512], pss[c][:],
                            bo_bc[:, c * 512:(c + 1) * 512])
                    nc.sync.dma_start(out_io[tj * NP:(tj + 1) * NP, :], fsb[:])

    nc.compile()
    return nc


def get_nc():
    if "nc" not in _CACHE:
        _CACHE["nc"] = _build()
    return _CACHE["nc"]


def make_in_maps(x, W_qkv, b_qkv, W_out, b_out):
    x = np.ascontiguousarray(np.asarray(x, dtype=np.float32))
    W_qkv = np.ascontiguousarray(np.asarray(W_qkv, dtype=np.float32))
    b_qkv = np.ascontiguousarray(np.asarray(b_qkv, dtype=np.float32))
    W_out = np.ascontiguousarray(np.asarray(W_out, dtype=np.float32))
    b_out = np.ascontiguousarray(np.asarray(b_out, dtype=np.float32))
    in_maps = []
    for core in range(NCORES):
        b, half = divmod(core, 2)
        xb = x[b]
        if half == 1:  # put this core's query rows first; key order is free
            xb = np.concatenate([xb[TQ:], xb[:TQ]], axis=0)
        in_maps.append({
            "x": np.ascontiguousarray(xb),
            "wqkv": W_qkv, "bqkv": b_qkv, "wout": W_out, "bout": b_out,
        })
    return in_maps


def run(in_maps, trace=False):
    from concourse.bass_utils import run_bass_kernel_spmd
    nc = get_nc()
    return run_bass_kernel_spmd(nc, in_maps, list(range(NCORES)), trace=trace)


def kernel(x, W_qkv, b_qkv, W_out, b_out):
    res = run(make_in_maps(x, W_qkv, b_qkv, W_out, b_out))
    out = np.empty((B, T, D), dtype=np.float32)
    for core in range(NCORES):
        b, half = divmod(core, 2)
        out[b, half * TQ:(half + 1) * TQ] = res.results[core]["out"]
    return out
